# revision 21
# baseline (speedup 1.0000x reference)
"""FP64->FP32 bit-circuit converter kernel for Trainium2 (8 NeuronCores).

Input:  fp64_pulse (1048576, 64) float32 of {0,1} bits (fp64, MSB first).
Output: (1048576, 32) float32 of {0,1} bits (fp32 conversion result).

Strategy (pure data parallel over batch, 131072 rows/core):
  - host packs the 64 {0,1}-floats of each row into the two 32-bit words
    of the IEEE-754 double they spell (hi = sign/exp/mant[0:20],
    lo = mant[20:52]) -- 8 bytes/row instead of 256 (32x less traffic),
  - DVE stock ops do the shift/mask field extraction,
  - six custom fused DVE ops (multi-ALU-stage uop programs) compute the
    bit-field combines, the round-to-nearest-even increment, the final
    exponent byte with overflow/underflow clamping, the masked mantissa
    field, and the NaN quiet-bit term -- each replacing several
    single-ALU instructions,
  - the final word is assembled with bitwise ORs of the disjoint field
    values; everything runs on the DVE in one chunk, so there are no
    cross-engine handoffs (the kernel is sync-latency-bound, not
    throughput-bound),
  - host unpacks the fp32 words back to the (B, 32) {0,1} float layout.

Engine semantics (hardware-probed):
  - DVE stock: shifts/bitwise exact on i32; arith/compares in fp32
    (exact below 2^24).  - Pool tensor_tensor: exact int32 (wraps).
  - Custom DVE uops: i32 ports value-convert to fp32 and back;
    arith/compare/min/max/select stages only (no shifts).
"""
import numpy as np

from concourse import bacc, mybir
from concourse import dve_ops
from concourse.dve_ops import DveOp, OPS, _SUB_OPCODE_FOR_NAME
from concourse.dve_spec import (Spec, Src0, Src1, C0, C1, C2, Zero, One,
                                lower, AluOp, Bin, minn, maxx, ne)
from concourse.dve_spec import _has_src1 as has_src1
from concourse.dve_uop import DveOpSpec
from concourse.tile import TileContext
from concourse.bass_utils import run_bass_kernel_spmd

AOT = mybir.AluOpType
AFT = mybir.ActivationFunctionType
I32 = mybir.dt.int32

B = 1_048_576
N_CORES = 8
B_CORE = B // N_CORES          # 131072
P = 128                        # partitions
NI = B_CORE // P               # 1024 inner rows per partition

_CACHE = {}


def _register(name, body):
    if name in _SUB_OPCODE_FOR_NAME:
        return next(o for o in OPS if o.name == name)
    spec = Spec(body=body)
    row = 1 + len(OPS)
    _SUB_OPCODE_FOR_NAME[name] = row
    ds = DveOpSpec(name=name, opcode=row, uops=lower(spec, ver="v3"),
                   rd1_en=has_src1(spec))
    op = DveOp(name, spec, False, {"v3": ds.sha("v3")})
    OPS.append(op)
    dve_ops.CUSTOM_DVE_SPECS[name] = spec
    return op


IS_GE, IS_GT, IS_EQ = AluOp.IS_GE, AluOp.IS_GT, AluOp.IS_EQ

# FP64_M: in0 = mS = S01*2^23 + m23, in1 = RL = L*2 + R.
#   M = m23 + (R & (S | L)) = m23 + (RL==3) + (RL==1)*S   [C0 = 2^23, C1 = 3]
_S = Bin(IS_GE, Src0, C0)
_m23v = Src0 - _S * C0
_ru = Bin(IS_EQ, Src1, C1) + Bin(IS_EQ, Src1, One) * _S
FP64_M_BODY = _m23v + _ru

# FP64_E8S: in0 = e, in1 = M.  [C0 = 896, C1 = 2^23, C2 = 255]
#   E8 = min((e >= 897) * (e - 896 + cry), 255); out = E8 << 23
_e2m = Src0 - C0
_cry = Bin(IS_GE, Src1, C1)
_ge = Bin(IS_GE, _e2m, One)
FP64_E8S_BODY = minn(_ge * (_e2m + _cry), C2) * C1

# FP64_MN: in0 = M, in1 = e.  [C0 = 2^23, C1 = 897, C2 = 1150]
#   mant = M - cry*2^23; nrm = (e>=897) - (e>1150); out = nrm * mant
_mant = Src0 - Bin(IS_GE, Src0, C0) * C0
_nrm = Bin(IS_GE, Src1, C1) - Bin(IS_GT, Src1, C2)
FP64_MN_BODY = _nrm * _mant

# FP64_NAN: in0 = e, in1 = anyv (any nonzero int32 <=> mantissa nonzero).
#   out = (e == 2047) * (anyv != 0) * 2^22   [C0 = 2047, C1 = 2^22]
FP64_NAN_BODY = Bin(IS_EQ, Src0, C0) * ne(Src1, Zero) * C1

# FP64_MAC: out = Src0 * C0 + Src1 (values < 2^24, fp32-exact)
FP64_MAC_BODY = Src0 * C0 + Src1

# FP64_MS: out = (Src0 != 0) * C0 + Src1  (sticky flag fold: in0 = s4 raw,
#   nonzero-ness survives the fp32 value cast; C0 = 2^23, in1 = m23)
FP64_MS_BODY = ne(Src0, Zero) * C0 + Src1

# FP64_NS: out = (Src0 < 0) * C0 + Src1  (sign term from raw hi: the fp32
#   value cast preserves sign; C0 = -2^31, in1 = nanv; -2^31, -2^31+2^22,
#   0, 2^22 are all fp32-exact)
FP64_NS_BODY = Bin(AluOp.IS_LT, Src0, Zero) * C0 + Src1

OP_M = _register("FP64_M", FP64_M_BODY)
OP_E8S = _register("FP64_E8S", FP64_E8S_BODY)
OP_MN = _register("FP64_MN", FP64_MN_BODY)
OP_NAN = _register("FP64_NAN", FP64_NAN_BODY)
OP_MAC = _register("FP64_MAC", FP64_MAC_BODY)
OP_MS = _register("FP64_MS", FP64_MS_BODY)
OP_NS = _register("FP64_NS", FP64_NS_BODY)


def _build(n_chunks=1):
    assert NI % n_chunks == 0
    CH = NI // n_chunks
    nc = bacc.Bacc("TRN2")
    hi_d = nc.dram_tensor("hi", [B_CORE, 1], I32, kind="ExternalInput")
    lo_d = nc.dram_tensor("lo", [B_CORE, 1], I32, kind="ExternalInput")
    y_d = nc.dram_tensor("y", [B_CORE, 1], I32, kind="ExternalOutput")

    hi_r = hi_d.ap().rearrange("(p n) d -> p (n d)", p=P)   # [128, 1024]
    lo_r = lo_d.ap().rearrange("(p n) d -> p (n d)", p=P)
    y_r = y_d.ap().rearrange("(p n) d -> p (n d)", p=P)

    with TileContext(nc) as tc:
        with (
            tc.tile_pool(name="io", bufs=2) as io,
            tc.tile_pool(name="sc", bufs=2) as sc,
        ):
            for ci in range(n_chunks):
                off = ci * CH

                def t(name):
                    return sc.tile([P, CH], I32, tag=name, name=name)[:, :]

                hin = io.tile([P, CH], I32, tag="hi", name="hin")
                lin = io.tile([P, CH], I32, tag="lo", name="lin")
                nc.sync.dma_start(hin[:, :], hi_r[:, off:off + CH])
                nc.sync.dma_start(lin[:, :], lo_r[:, off:off + CH])
                hi = hin[:, :]
                lo = lin[:, :]
                V, G, A = nc.vector, nc.gpsimd, nc.scalar

                # --- DVE stock: field extraction (shift/mask, exact) ---
                e = t("e")
                V.tensor_scalar(e, hi, 20, 0x7FF, AOT.logical_shift_right,
                                AOT.bitwise_and)
                mh8 = t("mh8")
                V.tensor_scalar(mh8, hi, 0xFFFFF, 3, AOT.bitwise_and,
                                AOT.logical_shift_left)
                lo29 = t("lo29")
                V.tensor_scalar(lo29, lo, 29, None, AOT.logical_shift_right)
                s4 = t("s4")
                V.tensor_scalar(s4, lo, 4, None, AOT.logical_shift_left)
                RL = t("RL")
                V.tensor_scalar(RL, lo, 28, 3, AOT.logical_shift_right,
                                AOT.bitwise_and)
                # m23 = mh8 + lo29 (disjoint bits, < 2^24: fp32-exact MAC)
                m23 = t("m23")
                V._custom_dve(OP_MAC, out=m23, in0=mh8, in1=lo29, s0=1.0)
                # mS = (s4 != 0)*2^23 + m23 (sticky flag folded in)
                mS = t("mS")
                V._custom_dve(OP_MS, out=mS, in0=s4, in1=m23, s0=8388608.0)
                # --- Pool side-chain (runs concurrently with the DVE
                # main chain; exact int32): nan quiet-bit + sign terms ---
                anyv = t("anyv")
                G.tensor_tensor(anyv, mS, RL, AOT.add)   # !=0 <=> mant != 0
                any01 = t("any01")
                G.tensor_scalar(any01, anyv, 1, None, AOT.is_ge)
                q2047 = t("q2047")
                G.tensor_scalar(q2047, e, 2047, None, AOT.is_equal)
                qa = t("qa")
                G.tensor_tensor(qa, q2047, any01, AOT.mult)
                nanv = t("nanv")
                G.tensor_scalar(nanv, qa, 4194304, None, AOT.mult)
                s01p = t("s01p")
                G.tensor_scalar(s01p, hi, 0, None, AOT.is_lt)
                sg = t("sg")
                G.tensor_scalar(sg, s01p, -2147483648.0, None, AOT.mult)
                ns = t("ns")
                G.tensor_tensor(ns, sg, nanv, AOT.add)

                # --- fused DVE ops ---
                M = t("M")
                V._custom_dve(OP_M, out=M, in0=mS, in1=RL,
                              s0=8388608.0, s1=3.0)
                E8s = t("E8s")
                V._custom_dve(OP_E8S, out=E8s, in0=e, in1=M,
                              s0=896.0, s1=8388608.0, imm2=255.0)
                Mn = t("Mn")
                V._custom_dve(OP_MN, out=Mn, in0=M, in1=e,
                              s0=8388608.0, s1=897.0, imm2=1150.0)

                # --- final assembly on DVE as bitwise ORs (fields are
                # disjoint: E8s bits 23-30, Mn bits 0-22, ns bit 31 + bit 22
                # only when Mn = 0). No cross-engine handoff at all. ---
                o1 = t("o1")
                V.tensor_tensor(o1, E8s, Mn, AOT.bitwise_or)
                out = t("out")
                V.tensor_tensor(out, o1, ns, AOT.bitwise_or)

                nc.sync.dma_start(y_r[:, off:off + CH], out)

    nc.compile()
    return nc


def _get_nc():
    if "nc" not in _CACHE:
        _CACHE["nc"] = _build()
    return _CACHE["nc"]


def _pack_inputs(x):
    """(B, 64) {0,1} float32 -> hi, lo int32 arrays of shape (B, 1)."""
    bits = x != 0
    pk = np.packbits(bits, axis=1)                  # (B, 8) MSB-first
    w = pk.view(">u4").astype(np.uint32)            # (B, 2) native
    hi = np.ascontiguousarray(w[:, 0]).view(np.int32).reshape(-1, 1)
    lo = np.ascontiguousarray(w[:, 1]).view(np.int32).reshape(-1, 1)
    return hi, lo


def _unpack_output(words):
    """(B, 1) int32 fp32 words -> (B, 32) float32 of {0,1} bits."""
    ob = words.reshape(-1).view(np.uint32).byteswap().view(np.uint8)
    return np.unpackbits(ob.reshape(-1, 4), axis=1).astype(np.float32)


def make_in_maps(x):
    hi, lo = _pack_inputs(np.ascontiguousarray(x, dtype=np.float32))
    return [
        {"hi": hi[c * B_CORE:(c + 1) * B_CORE],
         "lo": lo[c * B_CORE:(c + 1) * B_CORE]}
        for c in range(N_CORES)
    ]


def kernel(fp64_pulse: np.ndarray) -> np.ndarray:
    assert fp64_pulse.shape == (B, 64)
    nc = _get_nc()
    in_maps = make_in_maps(fp64_pulse)
    res = run_bass_kernel_spmd(nc, in_maps, core_ids=list(range(N_CORES)))
    words = np.concatenate([r["y"] for r in res.results], axis=0)
    return _unpack_output(words)
